# revision 10
# baseline (speedup 1.0000x reference)
"""DeformationGrid (trilinear interpolation) — TRN2 Bass kernel, v4.

The axon host<->device tunnel is ~60 MB/s, so wire bytes dominate wall time.
Design:
  - coords quantized to u16 on host (50 MB total, point-sharded on 8 cores)
  - theta quantized to int16 on host, sharded (12.6 MB total over the wire)
    and AllGathered on-device into the full c-last grid table
  - per point, two [P,1] indirect-DMA window fetches (one per x-plane of the
    brick) pull 390 contiguous int16 each: elements 0..5 hold the (dy=0)
    z-pair x 3 channels, 384..389 the (dy=1) pair.  One descriptor per
    partition is the only indirect-DMA shape the DGE executes exactly.
  - trilinear weights on ACT/DVE; output returned as int8 x24 (25 MB)
"""
import numpy as np
import concourse.bass as bass
import concourse.mybir as mybir
from concourse.alu_op_type import AluOpType
from concourse.tile import TileContext

P = 128
NQ = 256
TOTTILES = 32
CHUNK_TILES = (16, 8, 8)      # uneven chunks: smaller tail hides the last fetch
PPC = P * NQ * TOTTILES       # 1,048,576 points per core
PPT = P * NQ                  # points per core per tile
NB = 16                       # window batch (columns per extraction)
N_CORES = 8
GRID = 128
NC1 = GRID - 1                # 127
CELLS = GRID * GRID * GRID    # 2,097,152
NELEM = CELLS * 3             # i16 elements in the table
SLABR = CELLS // N_CORES      # 262,144 rows per core
WIN = 390                     # window: covers (j..j+1, k..k+1, 3ch)
F32 = mybir.dt.float32
F16 = mybir.dt.float16
I16 = mybir.dt.int16
I32 = mybir.dt.int32
I8 = mybir.dt.int8
U16 = mybir.dt.uint16
COPY = mybir.ActivationFunctionType.Copy
SV = np.float32(2048.0)       # int16 quantization scale for theta
OSCALE = np.float32(48.0)     # int8 quantization scale for the output
CSCALE = float(NC1) / 65535.0


def _split_sync_waits(nc, max_waits=1):
    # This container's walrus rejects >1 sync-wait per instruction; hoist
    # extras onto no-fuse NOPs placed just before the offender.
    ctr = [0]
    for f in nc.m.functions:
        for blk in f.blocks:
            out, changed = [], False
            for inst in blk.instructions:
                si = inst.sync_info
                waits = list(si.on_wait) if (si and si.on_wait) else []
                if len(waits) > max_waits:
                    changed = True
                    extra, keep = waits[:-max_waits], waits[-max_waits:]
                    for i in range(0, len(extra), max_waits):
                        ctr[0] += 1
                        out.append(mybir.InstNoOp(
                            name=f"waitsplit-{ctr[0]}", engine=inst.engine,
                            sync_info=mybir.SyncInfo(
                                on_wait=extra[i:i + max_waits], on_update=[]),
                            text_hint="waitsplit", bass_nofuse=True))
                    si.on_wait = keep
                out.append(inst)
            if changed:
                blk.instructions[:] = out


def _build_nc(TILES):
    nc = bass.Bass("TRN2", num_devices=N_CORES)
    qc = nc.dram_tensor("qc", [TILES, P, NQ * 3], U16, kind="ExternalInput")
    tslab = nc.dram_tensor("tslab", [SLABR, 3], I16, kind="ExternalInput")
    y = nc.dram_tensor("y", [TILES, P, NQ * 3], I8, kind="ExternalOutput")

    bounce = nc.dram_tensor("bounce", [SLABR, 3], I16, kind="Internal")
    tabs = nc.dram_tensor("tabs", [CELLS, 3], I16, kind="Internal",
                          addr_space="Shared")
    tab = nc.dram_tensor("tab", [NELEM, 1], I16, kind="Internal")

    with TileContext(nc) as tc:
        with (
            tc.tile_pool(name="io", bufs=2) as io,
            tc.tile_pool(name="win", bufs=2) as wpool,
            tc.tile_pool(name="mid", bufs=2) as mid,
        ):
            nc.sync.dma_start(bounce[:, :], tslab[:, :])
            nc.gpsimd.collective_compute(
                "AllGather", mybir.AluOpType.bypass,
                replica_groups=[list(range(N_CORES))],
                ins=[bounce[:, :]], outs=[tabs[:, :]])
            # gather source must be Local (and offset-0): copy out of Shared
            nc.sync.dma_start(
                tab[:, :].rearrange("(p f) o -> p (f o)", p=P),
                tabs[:, :].rearrange("(p f) c -> p (f c)", p=P))

            for t in range(TILES):
                ct = io.tile([P, NQ * 3], U16, tag="ct")
                nc.sync.dma_start(ct[:], qc[t, :, :])
                ctf = mid.tile([P, NQ * 3], F32, tag="ctf")
                nc.vector.tensor_copy(ctf[:], ct[:])
                c3 = ctf[:].rearrange("p (n c) -> p n c", c=3)

                # x' = u*127 - 0.5 ; i0 = RNE(x') == floor(u*127);
                # frac = (x' + 0.5) - i0
                xq, xi, fr = [], [], []
                for c in range(3):
                    q = mid.tile([P, NQ], F32, tag=f"xq{c}")
                    nc.scalar.activation(
                        q[:].rearrange("p (n o) -> p n o", o=1),
                        c3[:, :, c:c + 1], COPY, bias=-0.5, scale=CSCALE)
                    xq.append(q)
                for c in range(3):
                    i_ = mid.tile([P, NQ], I32, tag=f"xi{c}")
                    nc.vector.tensor_copy(i_[:], xq[c][:])
                    xi.append(i_)
                for c in range(3):
                    f_ = mid.tile([P, NQ], F32, tag=f"fr{c}")
                    nc.vector.scalar_tensor_tensor(
                        f_[:], xq[c][:], 0.5, xi[c][:],
                        AluOpType.add, AluOpType.subtract)
                    fr.append(f_)

                # start element = ((i*128 + j)*128 + k)*3, exact in f32
                cf1 = mid.tile([P, NQ], F32, tag="cf1")
                nc.vector.scalar_tensor_tensor(
                    cf1[:], xi[1][:], 128.0, xi[2][:],
                    AluOpType.mult, AluOpType.add)
                cf2 = mid.tile([P, NQ], F32, tag="cf2")
                nc.vector.scalar_tensor_tensor(
                    cf2[:], xi[0][:], 16384.0, cf1[:],
                    AluOpType.mult, AluOpType.add)
                cf3 = mid.tile([P, NQ], F32, tag="cf3")
                nc.vector.tensor_scalar_mul(cf3[:], cf2[:], 3.0)
                idx0 = mid.tile([P, NQ], I32, tag="idx0")
                nc.vector.tensor_copy(idx0[:], cf3[:])
                idx1 = mid.tile([P, NQ], I32, tag="idx1")
                nc.vector.tensor_scalar_add(idx1[:], idx0[:], 3 * GRID * GRID)

                # window fetches + extraction into vc [p, n, dd(4), zc(6)]
                vc = mid.tile([P, NQ * 24], I16, tag="vc")
                vc4 = vc[:].rearrange("p (n d e) -> p n d e", d=4, e=6)
                for b in range(NQ // NB):
                    w0 = wpool.tile([P, NB * WIN], I16, tag="w0")
                    w1 = wpool.tile([P, NB * WIN], I16, tag="w1")
                    for s in range(NB):
                        n = b * NB + s
                        nc.gpsimd.indirect_dma_start(
                            out=w0[:, s * WIN:(s + 1) * WIN], out_offset=None,
                            in_=tab[:, :],
                            in_offset=bass.IndirectOffsetOnAxis(
                                ap=idx0[:, n:n + 1], axis=0),
                            element_offset=0)
                        nc.gpsimd.indirect_dma_start(
                            out=w1[:, s * WIN:(s + 1) * WIN], out_offset=None,
                            in_=tab[:, :],
                            in_offset=bass.IndirectOffsetOnAxis(
                                ap=idx1[:, n:n + 1], axis=0),
                            element_offset=0)
                    # dy runs live at 0..5 and 384..389 inside each window
                    w0v = w0[:].rearrange("p (s w) -> p s w", w=WIN)
                    w1v = w1[:].rearrange("p (s w) -> p s w", w=WIN)
                    sl = slice(b * NB, (b + 1) * NB)
                    for dy in range(2):
                        nc.vector.tensor_copy(
                            vc4[:, sl, 0 + dy:1 + dy, :],
                            w0v[:, :, dy * 384:dy * 384 + 6].rearrange(
                                "p s (o w) -> p s o w", o=1))
                        nc.vector.tensor_copy(
                            vc4[:, sl, 2 + dy:3 + dy, :],
                            w1v[:, :, dy * 384:dy * 384 + 6].rearrange(
                                "p s (o w) -> p s o w", o=1))

                # xy corner weights a4 = {(1-fx)(1-fy), (1-fx)fy, fx(1-fy),
                # fx*fy} * sigma   (sigma carries the i16 dequant scale)
                fx, fy, fz = fr
                sigma = float(1.0 / float(SV))
                a4 = mid.tile([P, NQ * 4], F32, tag="a4")
                a4v = a4[:].rearrange("p (n j) -> p n j", j=4)
                fx1 = fx[:].rearrange("p (n o) -> p n o", o=1)
                fy1 = fy[:].rearrange("p (n o) -> p n o", o=1)
                nc.vector.scalar_tensor_tensor(
                    a4v[:, :, 3:4], fx1, sigma, fy1,
                    AluOpType.mult, AluOpType.mult)
                nc.vector.scalar_tensor_tensor(
                    a4v[:, :, 2:3], fx1, sigma, a4v[:, :, 3:4],
                    AluOpType.mult, AluOpType.subtract)
                nc.vector.scalar_tensor_tensor(
                    a4v[:, :, 1:2], fy1, sigma, a4v[:, :, 3:4],
                    AluOpType.mult, AluOpType.subtract)
                vtmp = mid.tile([P, NQ], F32, tag="vtmp")
                nc.vector.scalar_tensor_tensor(
                    vtmp[:].rearrange("p (n o) -> p n o", o=1), fx1, sigma,
                    a4v[:, :, 1:2], AluOpType.mult, AluOpType.add)
                nc.scalar.activation(
                    a4v[:, :, 0:1],
                    vtmp[:].rearrange("p (n o) -> p n o", o=1),
                    COPY, bias=sigma, scale=-1.0)

                # a4 order is {00, 01, 10, 11} over (dx, dy):
                #   a4[0]=(1-fx)(1-fy)  a4[1]=(1-fx)fy
                #   a4[2]=fx(1-fy)      a4[3]=fx*fy
                # matching vc's dd order (w0:dy0, w0:dy1, w1:dy0, w1:dy1).

                # z pair weights {1-fz, fz}
                wz = mid.tile([P, NQ * 2], F32, tag="wz")
                wzv = wz[:].rearrange("p (n z) -> p n z", z=2)
                fz1 = fz[:].rearrange("p (n o) -> p n o", o=1)
                nc.scalar.activation(wzv[:, :, 1:2], fz1, COPY,
                                     bias=0.0, scale=1.0)
                nc.scalar.activation(wzv[:, :, 0:1], fz1, COPY,
                                     bias=1.0, scale=-1.0)

                # W8[j, z] = a4[j] * wz[z]
                w8 = mid.tile([P, NQ * 8], F32, tag="w8")
                nc.vector.tensor_tensor(
                    w8[:].rearrange("p (n j z) -> p n j z", j=4, z=2),
                    a4v.rearrange("p n (j o) -> p n j o", o=1)
                       .to_broadcast([P, NQ, 4, 2]),
                    wzv.rearrange("p n (o z) -> p n o z", o=1)
                       .to_broadcast([P, NQ, 4, 2]),
                    AluOpType.mult)

                # combine: out_c = sum_r W8[r] * V[r, c]
                vc5 = vc[:].rearrange("p (n d z c) -> p n d z c",
                                      d=4, z=2, c=3)
                of = mid.tile([P, NQ * 3], F32, tag="of")
                o3 = of[:].rearrange("p (n c) -> p n c", c=3)
                for c in range(3):
                    p8 = mid.tile([P, NQ * 8], F32, tag="p8")
                    nc.vector.tensor_tensor(
                        p8[:].rearrange("p (n j z o) -> p n j z o",
                                        j=4, z=2, o=1),
                        w8[:].rearrange("p (n j z o) -> p n j z o",
                                        j=4, z=2, o=1),
                        vc5[:, :, :, :, c:c + 1], AluOpType.mult)
                    nc.vector.tensor_reduce(
                        out=o3[:, :, c:c + 1],
                        in_=p8[:].rearrange("p (n r) -> p n r", r=8),
                        axis=mybir.AxisListType.X, op=AluOpType.add)

                ot = io.tile([P, NQ * 3], I8, tag="ot")
                nc.vector.tensor_scalar_mul(ot[:], of[:], float(OSCALE))
                nc.sync.dma_start(y[t, :, :], ot[:])
    _split_sync_waits(nc)
    return nc


_CACHE = {}


def _get_runner():
    if "fns" in _CACHE:
        return _CACHE["fns"]
    import jax
    from jax.sharding import Mesh, PartitionSpec, NamedSharding
    from jax.experimental.shard_map import shard_map
    from concourse.bass2jax import _bass_exec_p, partition_id_tensor

    devices = jax.devices()[:N_CORES]
    mesh = Mesh(np.asarray(devices), ("core",))
    fns = {}
    for tiles in sorted(set(CHUNK_TILES)):
        nc = _build_nc(tiles)
        out_aval = jax.core.ShapedArray((tiles, P, NQ * 3), np.int8)
        pname = nc.partition_id_tensor.name if nc.partition_id_tensor else None
        in_names = ["qc", "tslab"] + ([pname] if pname else [])

        def _body(c, ts, nc=nc, out_aval=out_aval, in_names=tuple(in_names)):
            (out,) = _bass_exec_p.bind(
                c, ts, partition_id_tensor(),
                out_avals=(out_aval,), in_names=in_names,
                out_names=("y",), lowering_input_output_aliases=(),
                sim_require_finite=False, sim_require_nnan=False, nc=nc)
            return (out,)

        fns[tiles] = jax.jit(shard_map(_body, mesh=mesh,
                                       in_specs=(PartitionSpec("core"),) * 2,
                                       out_specs=(PartitionSpec("core"),),
                                       check_rep=False), keep_unused=True)
    _CACHE["sh"] = NamedSharding(mesh, PartitionSpec("core"))
    _CACHE["fns"] = fns
    return fns


def kernel(coords, theta):
    coords = np.asarray(coords, np.float32)
    theta = np.asarray(theta, np.float32)
    n = coords.shape[0]
    assert n == PPC * N_CORES, n

    fns = _get_runner()
    import jax
    sh = _CACHE["sh"]

    # theta uploads once; every chunk execution re-AllGathers it on-device.
    # The full-coords quantize runs inside theta's wire window.
    th = np.multiply(theta.reshape(CELLS, 3), SV).astype(np.int16)
    th_d = jax.device_put(th, sh)
    qall = np.multiply(coords, np.float32(65535.0)).astype(np.uint16) \
        .reshape(N_CORES, PPC, 3)

    # chunk k carries every core's k-th span: a complete 8-way shard
    outs = []
    off = 0
    for tiles in CHUNK_TILES:
        cpc = tiles * PPT
        qk = np.ascontiguousarray(qall[:, off:off + cpc])
        qd = jax.device_put(qk.reshape(N_CORES * tiles, P, NQ * 3), sh)
        ok = fns[tiles](qd, th_d)[0]
        try:
            ok.copy_to_host_async()
        except Exception:
            pass
        outs.append((off, cpc, ok))
        off += cpc

    res = np.empty((n, 3), np.float32)
    rv = res.reshape(N_CORES, PPC, 3)
    oinv = np.float32(1.0 / float(OSCALE))
    for off, cpc, ok in outs:
        np.multiply(np.asarray(ok).reshape(N_CORES, cpc, 3), oinv,
                    dtype=np.float32, out=rv[:, off:off + cpc])
    return res


# revision 11
# speedup vs baseline: 1.0099x; 1.0099x over previous
"""DeformationGrid (trilinear interpolation) — TRN2 Bass kernel, v4.

The axon host<->device tunnel is ~60 MB/s, so wire bytes dominate wall time.
Design:
  - coords quantized to u16 on host (50 MB total, point-sharded on 8 cores)
  - theta quantized to int16 on host, sharded (12.6 MB total over the wire)
    and AllGathered on-device into the full c-last grid table
  - per point, two [P,1] indirect-DMA window fetches (one per x-plane of the
    brick) pull 390 contiguous int16 each: elements 0..5 hold the (dy=0)
    z-pair x 3 channels, 384..389 the (dy=1) pair.  One descriptor per
    partition is the only indirect-DMA shape the DGE executes exactly.
  - trilinear weights on ACT/DVE; output returned as int8 x24 (25 MB)
"""
import numpy as np
import concourse.bass as bass
import concourse.mybir as mybir
from concourse.alu_op_type import AluOpType
from concourse.tile import TileContext

P = 128
NQ = 256
TILES = 16                    # tiles per chunk (program granularity)
CHUNKS = 2
PPC = P * NQ * TILES * CHUNKS  # 1,048,576 points per core
CPC = P * NQ * TILES          # 262,144 points per core per chunk
NB = 16                       # window batch (columns per extraction)
N_CORES = 8
GRID = 128
NC1 = GRID - 1                # 127
CELLS = GRID * GRID * GRID    # 2,097,152
NELEM = CELLS * 3             # i16 elements in the table
SLABR = CELLS // N_CORES      # 262,144 rows per core
WIN = 390                     # window: covers (j..j+1, k..k+1, 3ch)
F32 = mybir.dt.float32
F16 = mybir.dt.float16
I16 = mybir.dt.int16
I32 = mybir.dt.int32
I8 = mybir.dt.int8
U16 = mybir.dt.uint16
COPY = mybir.ActivationFunctionType.Copy
SV = np.float32(2048.0)       # int16 quantization scale for theta
OSCALE = np.float32(48.0)     # int8 quantization scale for the output
CSCALE = float(NC1) / 65535.0


def _split_sync_waits(nc, max_waits=1):
    # This container's walrus rejects >1 sync-wait per instruction; hoist
    # extras onto no-fuse NOPs placed just before the offender.
    ctr = [0]
    for f in nc.m.functions:
        for blk in f.blocks:
            out, changed = [], False
            for inst in blk.instructions:
                si = inst.sync_info
                waits = list(si.on_wait) if (si and si.on_wait) else []
                if len(waits) > max_waits:
                    changed = True
                    extra, keep = waits[:-max_waits], waits[-max_waits:]
                    for i in range(0, len(extra), max_waits):
                        ctr[0] += 1
                        out.append(mybir.InstNoOp(
                            name=f"waitsplit-{ctr[0]}", engine=inst.engine,
                            sync_info=mybir.SyncInfo(
                                on_wait=extra[i:i + max_waits], on_update=[]),
                            text_hint="waitsplit", bass_nofuse=True))
                    si.on_wait = keep
                out.append(inst)
            if changed:
                blk.instructions[:] = out


def _build_nc():
    nc = bass.Bass("TRN2", num_devices=N_CORES)
    qc = nc.dram_tensor("qc", [TILES, P, NQ * 3], U16, kind="ExternalInput")
    tslab = nc.dram_tensor("tslab", [SLABR, 3], I16, kind="ExternalInput")
    y = nc.dram_tensor("y", [TILES, P, NQ * 3], I8, kind="ExternalOutput")

    bounce = nc.dram_tensor("bounce", [SLABR, 3], I16, kind="Internal")
    tabs = nc.dram_tensor("tabs", [CELLS, 3], I16, kind="Internal",
                          addr_space="Shared")
    tab = nc.dram_tensor("tab", [NELEM, 1], I16, kind="Internal")

    with TileContext(nc) as tc:
        with (
            tc.tile_pool(name="io", bufs=2) as io,
            tc.tile_pool(name="win", bufs=2) as wpool,
            tc.tile_pool(name="mid", bufs=2) as mid,
        ):
            nc.sync.dma_start(bounce[:, :], tslab[:, :])
            nc.gpsimd.collective_compute(
                "AllGather", mybir.AluOpType.bypass,
                replica_groups=[list(range(N_CORES))],
                ins=[bounce[:, :]], outs=[tabs[:, :]])
            # gather source must be Local (and offset-0): copy out of Shared
            nc.sync.dma_start(
                tab[:, :].rearrange("(p f) o -> p (f o)", p=P),
                tabs[:, :].rearrange("(p f) c -> p (f c)", p=P))

            for t in range(TILES):
                ct = io.tile([P, NQ * 3], U16, tag="ct")
                nc.sync.dma_start(ct[:], qc[t, :, :])
                ctf = mid.tile([P, NQ * 3], F32, tag="ctf")
                nc.vector.tensor_copy(ctf[:], ct[:])
                c3 = ctf[:].rearrange("p (n c) -> p n c", c=3)

                # x' = u*127 - 0.5 ; i0 = RNE(x') == floor(u*127);
                # frac = (x' + 0.5) - i0
                xq, xi, fr = [], [], []
                for c in range(3):
                    q = mid.tile([P, NQ], F32, tag=f"xq{c}")
                    nc.scalar.activation(
                        q[:].rearrange("p (n o) -> p n o", o=1),
                        c3[:, :, c:c + 1], COPY, bias=-0.5, scale=CSCALE)
                    xq.append(q)
                for c in range(3):
                    i_ = mid.tile([P, NQ], I32, tag=f"xi{c}")
                    nc.vector.tensor_copy(i_[:], xq[c][:])
                    xi.append(i_)
                for c in range(3):
                    f_ = mid.tile([P, NQ], F32, tag=f"fr{c}")
                    nc.vector.scalar_tensor_tensor(
                        f_[:], xq[c][:], 0.5, xi[c][:],
                        AluOpType.add, AluOpType.subtract)
                    fr.append(f_)

                # start element = ((i*128 + j)*128 + k)*3, exact in f32
                cf1 = mid.tile([P, NQ], F32, tag="cf1")
                nc.vector.scalar_tensor_tensor(
                    cf1[:], xi[1][:], 128.0, xi[2][:],
                    AluOpType.mult, AluOpType.add)
                cf2 = mid.tile([P, NQ], F32, tag="cf2")
                nc.vector.scalar_tensor_tensor(
                    cf2[:], xi[0][:], 16384.0, cf1[:],
                    AluOpType.mult, AluOpType.add)
                cf3 = mid.tile([P, NQ], F32, tag="cf3")
                nc.vector.tensor_scalar_mul(cf3[:], cf2[:], 3.0)
                idx0 = mid.tile([P, NQ], I32, tag="idx0")
                nc.vector.tensor_copy(idx0[:], cf3[:])
                idx1 = mid.tile([P, NQ], I32, tag="idx1")
                nc.vector.tensor_scalar_add(idx1[:], idx0[:], 3 * GRID * GRID)

                # window fetches + extraction into vc [p, n, dd(4), zc(6)]
                vc = mid.tile([P, NQ * 24], I16, tag="vc")
                vc4 = vc[:].rearrange("p (n d e) -> p n d e", d=4, e=6)
                for b in range(NQ // NB):
                    w0 = wpool.tile([P, NB * WIN], I16, tag="w0")
                    w1 = wpool.tile([P, NB * WIN], I16, tag="w1")
                    for s in range(NB):
                        n = b * NB + s
                        nc.gpsimd.indirect_dma_start(
                            out=w0[:, s * WIN:(s + 1) * WIN], out_offset=None,
                            in_=tab[:, :],
                            in_offset=bass.IndirectOffsetOnAxis(
                                ap=idx0[:, n:n + 1], axis=0),
                            element_offset=0)
                        nc.gpsimd.indirect_dma_start(
                            out=w1[:, s * WIN:(s + 1) * WIN], out_offset=None,
                            in_=tab[:, :],
                            in_offset=bass.IndirectOffsetOnAxis(
                                ap=idx1[:, n:n + 1], axis=0),
                            element_offset=0)
                    # dy runs live at 0..5 and 384..389 inside each window
                    w0v = w0[:].rearrange("p (s w) -> p s w", w=WIN)
                    w1v = w1[:].rearrange("p (s w) -> p s w", w=WIN)
                    sl = slice(b * NB, (b + 1) * NB)
                    for dy in range(2):
                        nc.vector.tensor_copy(
                            vc4[:, sl, 0 + dy:1 + dy, :],
                            w0v[:, :, dy * 384:dy * 384 + 6].rearrange(
                                "p s (o w) -> p s o w", o=1))
                        nc.vector.tensor_copy(
                            vc4[:, sl, 2 + dy:3 + dy, :],
                            w1v[:, :, dy * 384:dy * 384 + 6].rearrange(
                                "p s (o w) -> p s o w", o=1))

                # xy corner weights a4 = {(1-fx)(1-fy), (1-fx)fy, fx(1-fy),
                # fx*fy} * sigma   (sigma carries the i16 dequant scale)
                fx, fy, fz = fr
                sigma = float(1.0 / float(SV))
                a4 = mid.tile([P, NQ * 4], F32, tag="a4")
                a4v = a4[:].rearrange("p (n j) -> p n j", j=4)
                fx1 = fx[:].rearrange("p (n o) -> p n o", o=1)
                fy1 = fy[:].rearrange("p (n o) -> p n o", o=1)
                nc.vector.scalar_tensor_tensor(
                    a4v[:, :, 3:4], fx1, sigma, fy1,
                    AluOpType.mult, AluOpType.mult)
                nc.vector.scalar_tensor_tensor(
                    a4v[:, :, 2:3], fx1, sigma, a4v[:, :, 3:4],
                    AluOpType.mult, AluOpType.subtract)
                nc.vector.scalar_tensor_tensor(
                    a4v[:, :, 1:2], fy1, sigma, a4v[:, :, 3:4],
                    AluOpType.mult, AluOpType.subtract)
                vtmp = mid.tile([P, NQ], F32, tag="vtmp")
                nc.vector.scalar_tensor_tensor(
                    vtmp[:].rearrange("p (n o) -> p n o", o=1), fx1, sigma,
                    a4v[:, :, 1:2], AluOpType.mult, AluOpType.add)
                nc.scalar.activation(
                    a4v[:, :, 0:1],
                    vtmp[:].rearrange("p (n o) -> p n o", o=1),
                    COPY, bias=sigma, scale=-1.0)

                # a4 order is {00, 01, 10, 11} over (dx, dy):
                #   a4[0]=(1-fx)(1-fy)  a4[1]=(1-fx)fy
                #   a4[2]=fx(1-fy)      a4[3]=fx*fy
                # matching vc's dd order (w0:dy0, w0:dy1, w1:dy0, w1:dy1).

                # z pair weights {1-fz, fz}
                wz = mid.tile([P, NQ * 2], F32, tag="wz")
                wzv = wz[:].rearrange("p (n z) -> p n z", z=2)
                fz1 = fz[:].rearrange("p (n o) -> p n o", o=1)
                nc.scalar.activation(wzv[:, :, 1:2], fz1, COPY,
                                     bias=0.0, scale=1.0)
                nc.scalar.activation(wzv[:, :, 0:1], fz1, COPY,
                                     bias=1.0, scale=-1.0)

                # W8[j, z] = a4[j] * wz[z]
                w8 = mid.tile([P, NQ * 8], F32, tag="w8")
                nc.vector.tensor_tensor(
                    w8[:].rearrange("p (n j z) -> p n j z", j=4, z=2),
                    a4v.rearrange("p n (j o) -> p n j o", o=1)
                       .to_broadcast([P, NQ, 4, 2]),
                    wzv.rearrange("p n (o z) -> p n o z", o=1)
                       .to_broadcast([P, NQ, 4, 2]),
                    AluOpType.mult)

                # combine: out_c = sum_r W8[r] * V[r, c]
                vc5 = vc[:].rearrange("p (n d z c) -> p n d z c",
                                      d=4, z=2, c=3)
                of = mid.tile([P, NQ * 3], F32, tag="of")
                o3 = of[:].rearrange("p (n c) -> p n c", c=3)
                for c in range(3):
                    p8 = mid.tile([P, NQ * 8], F32, tag="p8")
                    nc.vector.tensor_tensor(
                        p8[:].rearrange("p (n j z o) -> p n j z o",
                                        j=4, z=2, o=1),
                        w8[:].rearrange("p (n j z o) -> p n j z o",
                                        j=4, z=2, o=1),
                        vc5[:, :, :, :, c:c + 1], AluOpType.mult)
                    nc.vector.tensor_reduce(
                        out=o3[:, :, c:c + 1],
                        in_=p8[:].rearrange("p (n r) -> p n r", r=8),
                        axis=mybir.AxisListType.X, op=AluOpType.add)

                ot = io.tile([P, NQ * 3], I8, tag="ot")
                nc.vector.tensor_scalar_mul(ot[:], of[:], float(OSCALE))
                nc.sync.dma_start(y[t, :, :], ot[:])
    _split_sync_waits(nc)
    return nc


_CACHE = {}


def _get_runner():
    if "fn" in _CACHE:
        return _CACHE["fn"]
    import jax
    from jax.sharding import Mesh, PartitionSpec
    from jax.experimental.shard_map import shard_map
    from concourse.bass2jax import _bass_exec_p, partition_id_tensor

    nc = _build_nc()
    devices = jax.devices()[:N_CORES]
    mesh = Mesh(np.asarray(devices), ("core",))
    out_aval = jax.core.ShapedArray((TILES, P, NQ * 3), np.int8)
    pname = nc.partition_id_tensor.name if nc.partition_id_tensor else None
    in_names = ["qc", "tslab"] + ([pname] if pname else [])

    def _body(c, ts):
        (out,) = _bass_exec_p.bind(
            c, ts, partition_id_tensor(),
            out_avals=(out_aval,), in_names=tuple(in_names),
            out_names=("y",), lowering_input_output_aliases=(),
            sim_require_finite=False, sim_require_nnan=False, nc=nc)
        return (out,)

    fn = jax.jit(shard_map(_body, mesh=mesh,
                           in_specs=(PartitionSpec("core"),) * 2,
                           out_specs=(PartitionSpec("core"),),
                           check_rep=False), keep_unused=True)
    from jax.sharding import NamedSharding
    _CACHE["sh"] = NamedSharding(mesh, PartitionSpec("core"))
    _CACHE["fn"] = fn
    return fn


def kernel(coords, theta):
    coords = np.asarray(coords, np.float32)
    theta = np.asarray(theta, np.float32)
    n = coords.shape[0]
    assert n == PPC * N_CORES, n

    fn = _get_runner()
    import jax
    sh = _CACHE["sh"]

    # theta uploads once; every chunk execution re-AllGathers it on-device.
    # The full-coords quantize runs inside theta's wire window.
    th = np.multiply(theta.reshape(CELLS, 3), SV).astype(np.int16)
    th_d = jax.device_put(th, sh)
    qall = np.multiply(coords, np.float32(65535.0)).astype(np.uint16) \
        .reshape(N_CORES, CHUNKS, CPC, 3)

    # chunk k carries every core's k-th quarter: a complete 8-way shard
    outs = []
    for k in range(CHUNKS):
        qk = np.ascontiguousarray(qall[:, k])
        qd = jax.device_put(qk.reshape(N_CORES * TILES, P, NQ * 3), sh)
        ok = fn(qd, th_d)[0]
        try:
            ok.copy_to_host_async()
        except Exception:
            pass
        outs.append(ok)

    res = np.empty((n, 3), np.float32)
    rv = res.reshape(N_CORES, CHUNKS, CPC, 3)
    oinv = np.float32(1.0 / float(OSCALE))
    for k, ok in enumerate(outs):
        np.multiply(np.asarray(ok).reshape(N_CORES, CPC, 3), oinv,
                    dtype=np.float32, out=rv[:, k])
    return res
